# revision 49
# baseline (speedup 1.0000x reference)
"""Multi-head causal attention (QKV proj + RoPE + softmax attention + out proj)
as a distributed Bass kernel on 8 Trainium2 NeuronCores.

Sharding: tensor-parallel over heads (2 of 16 heads per core). Each core
computes Q/K/V for its heads from the replicated input, runs attention, and
AllGathers per-head attention outputs (d-major) so every core can compute a
256-column slice of the output projection. The host concatenates the slices.

v2 schedule: work is decomposed into 8 (batch, 512-token q-chunk) attention
units, each followed by its own small AllGather, woven between the QKV
projection chunks as soon as their K/V dependencies exist. Output-projection
units consume gathered chunks a few chunks later, so collectives always
complete long before their consumers and the tail after the last attention
unit is just one small AllGather + one outproj unit. The out projection runs
wo-stationary streaming 512 tokens per matmul into an [oc, token] PSUM; the
host transposes the final [DQ, NT] result.
"""

import math
import numpy as np
import ml_dtypes

B, S, D, H = 2, 2048, 2048, 16
HD = 128                  # head dim
P = 128                   # SBUF partitions
NT = B * S                # 4096 tokens
N_CORES = 8
HPC = H // N_CORES        # heads per core
DQ = HPC * HD             # 256 q/k/v rows per core
KC = D // P               # 16 contraction chunks
TCH = 512                 # token chunk in QKV projection
NTC = NT // TCH           # 8
SBK = S // P              # 16 key blocks per batch
QCH = 512                 # q tokens per attention unit
BF = ml_dtypes.bfloat16

_cache = {}


def _vaug_col(b, i, h):
    # column base of V chunk (batch b, s-chunk i, head h) in the vaug tile
    return ((b * SBK + i) * HPC + h) * (HD + 1)


def _build(mask_mode):
    from concourse import bacc
    import concourse.mybir as mybir
    import concourse.tile as tile
    from concourse.tile_rust import add_dep_helper

    bf = mybir.dt.bfloat16
    f32 = mybir.dt.float32
    EXP = mybir.ActivationFunctionType.Exp
    CPY = mybir.ActivationFunctionType.Copy
    scale = 1.0 / math.sqrt(HD)
    causal = mask_mode == "causal"

    nc = bacc.Bacc("TRN2", target_bir_lowering=False, debug=False,
                   num_devices=N_CORES)

    xT = nc.declare_dram_parameter("xT", [D, NT], bf, isOutput=False)
    wqT = nc.declare_dram_parameter("wqT", [D, DQ], bf, isOutput=False)
    wkT = nc.declare_dram_parameter("wkT", [D, DQ], bf, isOutput=False)
    wvT = nc.declare_dram_parameter("wvT", [D, DQ], bf, isOutput=False)
    woT = nc.declare_dram_parameter("woT", [D, DQ], bf, isOutput=False)
    cro = nc.declare_dram_parameter("cro", [P, S], bf, isOutput=False)
    sro = nc.declare_dram_parameter("sro", [P, S], bf, isOutput=False)
    cst = nc.declare_dram_parameter("cst", [P, 3 * P], bf, isOutput=False)
    mskT = None
    if mask_mode == "general":
        mskT = nc.declare_dram_parameter("mskT", [S, S], bf, isOutput=False)
    # output is [oc, token]; the host transposes
    out = nc.declare_dram_parameter("out", [DQ, NT], f32, isOutput=True)

    rg = [list(range(N_CORES))]

    with tile.TileContext(nc) as tc:
        with (
            tc.tile_pool(name="per", bufs=1) as per,
            tc.tile_pool(name="stage", bufs=8) as stage,
            tc.tile_pool(name="ost", bufs=3) as ostp,
            tc.tile_pool(name="dram", bufs=1, space="DRAM") as drp,
            tc.tile_pool(name="ptp", bufs=2) as ptp,
            tc.tile_pool(name="ags", bufs=12) as ags,
            tc.tile_pool(name="mkp",
                         bufs=(4 if mask_mode == "general" else 1)) as mkp,
            tc.tile_pool(name="ps_qk", bufs=4, space="PSUM") as ps_qk,
            tc.tile_pool(name="ps_v", bufs=2, space="PSUM") as ps_v,
            tc.tile_pool(name="ps_at", bufs=2, space="PSUM") as ps_at,
            tc.tile_pool(name="xs", bufs=12) as xs,
            tc.tile_pool(name="rt", bufs=4) as rt,
        ):
            # ---------------- persistent SBUF ----------------
            q_sb = per.tile([P, HPC * NT], bf)       # d-major Q, head h at h*NT
            k_sb = per.tile([P, HPC * NT], bf)
            vaug_sb = per.tile([P, B * SBK * HPC * (HD + 1)], bf)
            attn_sb = per.tile([P, HPC * NT], bf)    # d-major attention out
            wo_sb = per.tile([P, KC * DQ], bf)
            cst_sb = per.tile([P, 3 * P], bf)
            ident = cst_sb[:, 0:P]
            perm = cst_sb[:, P:2 * P]
            tri01 = cst_sb[:, 2 * P:3 * P]

            nc.sync.dma_start(out=cst_sb[:], in_=cst[:, :])
            # ones columns for the PV denominator trick
            nc.gpsimd.memset(vaug_sb[:], 1.0)

            # projection-scoped SBUF
            wq_sb, free_wq = tc.tile([P, KC * DQ], bf, name="wq_sb")
            wk_sb, free_wk = tc.tile([P, KC * DQ], bf, name="wk_sb")
            wv_sb, free_wv = tc.tile([P, KC * DQ], bf, name="wv_sb")
            cro_sb, free_cro = tc.tile([P, S], bf, name="cro_sb")
            sro_sb, free_sro = tc.tile([P, S], bf, name="sro_sb")

            # DRAM bounce + gathered buffers. Early units share merged
            # collectives (fewer AllGathers -> the scheduler's pessimistic
            # collective cost model serializes less); the final unit is
            # split into two 256-token halves to shrink the exposed tail.
            agw = {"00": 2 * QCH, "01": 2 * QCH, "10": 2 * QCH,
                   "12": QCH, "3a": QCH // 2, "3b": QCH // 2}
            bounce = {k: drp.tile([DQ, w], bf, name=f"bounce{k}")
                      for k, w in agw.items()}
            ag = {k: drp.tile([D, w], bf, addr_space="Shared",
                              name=f"ag{k}") for k, w in agw.items()}

            last_xt = [None]   # most recent xt DMA instruction (ordering aid)
            last_pv = [None]   # most recent PV matmul (tail-ordering aid)

            def fire_ag(key):
                nc.gpsimd.collective_compute(
                    "AllGather", mybir.AluOpType.bypass,
                    replica_groups=rg,
                    ins=[bounce[key].opt()], outs=[ag[key].opt()])

            def attn_unit(b, qc, bkey, boff, jj_range=None, nbounce=2,
                          pts=None, ret_pts=False):
                n_s = 4 * qc + 4 if causal else SBK
                if pts is None:
                    pts = [ptp.tile([P, n_s * QCH], bf, tag="pt",
                                    name=f"pt{b}{qc}{h}")
                           for h in range(HPC)]
                    do_st = True
                else:
                    do_st = False   # ST phase already done by the first half
                for h in range(HPC):
                    qoff = h * NT + b * S + qc * QCH
                    pt = pts[h]
                    if not do_st:
                        continue
                    for sb in range(n_s):
                        # diagonal blocks: only q >= key-block start is live
                        off = max(0, (sb - 4 * qc) * P) if causal else 0
                        stp = ps_at.tile([P, QCH], f32, tag="at",
                                         name=f"st{b}{qc}{h}{sb}")
                        nc.tensor.matmul(
                            stp[:, 0:QCH - off],
                            k_sb[:, h * NT + b * S + sb * P:
                                 h * NT + b * S + (sb + 1) * P],
                            q_sb[:, qoff + off:qoff + QCH])
                        if mask_mode == "general":
                            mk = mkp.tile([P, QCH], bf, tag="mk",
                                          name=f"mk{b}{qc}{h}{sb}")
                            nc.sync.dma_start(
                                out=mk[:],
                                in_=mskT[sb * P:(sb + 1) * P,
                                         qc * QCH:(qc + 1) * QCH])
                            nc.vector.tensor_add(stp[:], stp[:], mk[:])
                        nc.scalar.activation(
                            pt[:, sb * QCH + off:(sb + 1) * QCH],
                            stp[:, 0:QCH - off], EXP, scale=scale)
                        if causal and sb >= 4 * qc:
                            j = sb - 4 * qc
                            c0 = sb * QCH + j * P
                            nc.vector.tensor_mul(
                                pt[:, c0:c0 + P], pt[:, c0:c0 + P], tri01)
                for jj in (range(QCH // P) if jj_range is None
                           else range(*jj_range)):
                    for h in range(HPC):
                        qoff = h * NT + b * S + qc * QCH
                        pt = pts[h]
                        qb = 4 * qc + jj
                        n_pv = qb + 1 if causal else SBK
                        pv = ps_at.tile([P, HD + 1], f32, tag="at",
                                        name=f"pv{b}{qc}{h}{jj}")
                        for sb in range(n_pv):
                            last_pv[0] = nc.tensor.matmul(
                                pv[:],
                                pt[:, sb * QCH + jj * P:sb * QCH + (jj + 1) * P],
                                vaug_sb[:, _vaug_col(b, sb, h):
                                        _vaug_col(b, sb, h) + HD + 1],
                                start=(sb == 0), stop=(sb == n_pv - 1))
                        rec = stage.tile([P, 1], f32, tag="rec",
                                         name=f"rec{b}{qc}{h}{jj}")
                        nc.vector.reciprocal(rec[:], pv[:, HD:HD + 1])
                        ast = stage.tile([P, P], bf, tag="ast",
                                         name=f"ast{b}{qc}{h}{jj}")
                        nc.vector.tensor_scalar_mul(ast[:], pv[:, 0:HD], rec[:])
                        trp = ps_at.tile([P, P], bf, tag="at",
                                         name=f"tr{b}{qc}{h}{jj}")
                        nc.tensor.transpose(trp[:], ast[:], ident)
                        nc.vector.tensor_copy(
                            attn_sb[:, qoff + jj * P:qoff + (jj + 1) * P],
                            trp[:])
                # bounce this unit's tokens into the (possibly shared) buffer
                jlo = 0 if jj_range is None else jj_range[0]
                jhi = QCH // P if jj_range is None else jj_range[1]
                w = (jhi - jlo) * P
                for h in range(HPC):
                    base = h * NT + b * S + qc * QCH + jlo * P
                    for u in range(nbounce):
                        o = u * (w // nbounce)
                        nc.gpsimd.dma_start(
                            out=bounce[bkey][h * HD:(h + 1) * HD,
                                             boff + o:boff + o + w // nbounce],
                            in_=attn_sb[:, base + o:base + o + w // nbounce])
                return pts if ret_pts else None

            def outproj_unit(akey, t0, coff, w, tag, split_q=False,
                             hold_back=False, engs=None):
                # gathered activations: batched loads ([128, 2, w] pairs, or
                # finer per-kk loads when the load latency is tail-critical),
                # issued on the given queue list (round-robin) so the tail
                # units don't serialize behind the sync sequencer
                agts = []
                if split_q:
                    for kk in range(KC):
                        agt = ags.tile([P, w], bf, tag="agt",
                                       name=f"agt{tag}{kk}")
                        src = ag[akey][kk * P:(kk + 1) * P, coff:coff + w]
                        eng = engs[kk % len(engs)] if engs else \
                            (nc.scalar if kk % 2 else nc.sync)
                        eng.dma_start(out=agt[:], in_=src)
                        agts.append(agt)
                else:
                    for k2 in range(KC // 2):
                        agt = ags.tile([P, 2, w], bf, tag="agt",
                                       name=f"agt{tag}{k2}")
                        src = ag[akey][k2 * 2 * P:(k2 + 1) * 2 * P,
                                       coff:coff + w]
                        src = src.rearrange("(k p) c -> p k c", p=P)
                        eng = engs[k2 % len(engs)] if engs else nc.sync
                        mm = eng.dma_start(out=agt[:], in_=src)
                        if engs is None and last_xt[0] is not None:
                            # never let gather-gated loads get scheduled
                            # ahead of projection-critical input loads in
                            # the sync queue
                            add_dep_helper(mm.ins, last_xt[0].ins, sync=False,
                                           reason="agt after xt")
                        agts.append(agt)
                ops = [ps_v.tile([P, QCH], f32, tag="vps",
                                 name=f"op{tag}{ob}") for ob in range(2)]
                for kk in range(KC):
                    rhs = (agts[kk][:, :] if split_q
                           else agts[kk // 2][:, kk % 2, :])
                    for ob in range(2):
                        mm = nc.tensor.matmul(
                            ops[ob][:, 0:w],
                            wo_sb[:, kk * DQ + ob * P:kk * DQ + (ob + 1) * P],
                            rhs, start=(kk == 0), stop=(kk == KC - 1))
                        if hold_back and kk == 0 and ob == 0 \
                                and last_pv[0] is not None:
                            # keep this unit's matmuls queued behind the
                            # final attention unit so they fill the PE
                            # during the last AllGather's latency window
                            add_dep_helper(mm.ins, last_pv[0].ins, sync=False,
                                           reason="tail filler ordering")
                for ob in range(2):
                    ost = ostp.tile([P, QCH], f32, tag="ost",
                                    name=f"ost{tag}{ob}")
                    nc.vector.tensor_copy(ost[:, 0:w], ops[ob][:, 0:w])
                    nc.scalar.dma_start(
                        out=out[ob * P:(ob + 1) * P, t0:t0 + w],
                        in_=ost[:, 0:w])

            def chunk(tci):
                t0 = tci * TCH
                qp = [ps_qk.tile([P, TCH], f32, tag="qkps", name=f"qp{tci}_{m}")
                      for m in range(HPC)]
                kp = [ps_qk.tile([P, TCH], f32, tag="qkps", name=f"kp{tci}_{m}")
                      for m in range(HPC)]
                vp = [ps_v.tile([P, 2 * DQ], f32, tag="vps",
                                name=f"vp{tci}_{u}") for u in range(2)]
                vfirst = {}
                for kk in range(KC):
                    if tci == 0:
                        if kk == 0:
                            # rope tables first, 4-way split for ring
                            # parallelism: needed by the chunk-0 rope
                            for u in range(4):
                                cs = slice(u * S // 4, (u + 1) * S // 4)
                                nc.gpsimd.dma_start(out=cro_sb[:, cs],
                                                    in_=cro[:, cs])
                                nc.gpsimd.dma_start(out=sro_sb[:, cs],
                                                    in_=sro[:, cs])
                        # stream weights in just ahead of first use, spread
                        # over all three issue queues so no single sequencer
                        # rate-limits the first chunk
                        nc.scalar.dma_start(
                            out=wq_sb[:, kk * DQ:(kk + 1) * DQ],
                            in_=wqT[kk * P:(kk + 1) * P, :])
                        nc.sync.dma_start(
                            out=wk_sb[:, kk * DQ:(kk + 1) * DQ],
                            in_=wkT[kk * P:(kk + 1) * P, :])
                        nc.gpsimd.dma_start(
                            out=wv_sb[:, kk * DQ:(kk + 1) * DQ],
                            in_=wvT[kk * P:(kk + 1) * P, :])
                    xt = xs.tile([P, TCH], bf, tag="xt", name=f"xt{tci}_{kk}")
                    last_xt[0] = nc.sync.dma_start(
                        out=xt[:], in_=xT[kk * P:(kk + 1) * P, t0:t0 + TCH])
                    st = (kk == 0)
                    sp = (kk == KC - 1)
                    for m in range(HPC):
                        nc.tensor.matmul(
                            qp[m], wq_sb[:, kk * DQ + m * HD:kk * DQ + (m + 1) * HD],
                            xt[:], start=st, stop=sp)
                        nc.tensor.matmul(
                            kp[m], wk_sb[:, kk * DQ + m * HD:kk * DQ + (m + 1) * HD],
                            xt[:], start=st, stop=sp)
                    for tb in range(TCH // P):
                        mm = nc.tensor.matmul(
                            vp[tb // 2][:, (tb % 2) * DQ:(tb % 2 + 1) * DQ],
                            xt[:, tb * P:(tb + 1) * P],
                            wv_sb[:, kk * DQ:(kk + 1) * DQ],
                            start=(st and tb % 2 == 0), stop=sp,
                            skip_group_check=(tb % 2 == 1))
                        if kk == 0:
                            vfirst[tb] = mm

                for u in range(2):
                    # the second group's first MM must follow the bank
                    # clear done by the first group's start=True MM
                    add_dep_helper(vfirst[u * 2 + 1].ins, vfirst[u * 2].ins,
                                   sync=False,
                                   reason="bank-clear before 2nd V group")
                # V: copy token-major psum into vaug (both heads per op)
                for tb in range(TCH // P):
                    tglob = t0 + tb * P
                    b = tglob // S
                    i = (tglob % S) // P
                    c0 = _vaug_col(b, i, 0)
                    dst = vaug_sb[:, c0:c0 + HPC * (HD + 1)]
                    dst = dst.rearrange("p (h d) -> p h d", h=HPC)[:, :, 0:HD]
                    src = vp[tb // 2][:, (tb % 2) * DQ:(tb % 2 + 1) * DQ]
                    src = src.rearrange("p (h d) -> p h d", h=HPC)
                    nc.vector.tensor_copy(dst, src)
                # RoPE on Q and K (d-major): out = C*z + Sro*pairswap(z)
                t0s = t0 % S
                for (ps_list, dst) in ((qp, q_sb), (kp, k_sb)):
                    for m in range(HPC):
                        zb = stage.tile([P, TCH], bf, tag="zb",
                                        name=f"zb{tci}{m}")
                        # on DVE, not scalar: keeps the scalar engine free
                        # for the attention exp chains
                        nc.vector.tensor_copy(zb[:], ps_list[m][:])
                        zs = ps_qk.tile([P, TCH], f32, tag="qkps",
                                        name=f"zs{tci}{m}")
                        nc.tensor.matmul(zs[:], perm, zb[:])
                        t1 = rt.tile([P, TCH], f32, tag="t1", name=f"t1{tci}{m}")
                        t2 = rt.tile([P, TCH], f32, tag="t2", name=f"t2{tci}{m}")
                        nc.vector.tensor_mul(t1[:], zb[:],
                                             cro_sb[:, t0s:t0s + TCH])
                        nc.vector.tensor_mul(t2[:], zs[:],
                                             sro_sb[:, t0s:t0s + TCH])
                        nc.vector.tensor_add(
                            dst[:, m * NT + t0:m * NT + t0 + TCH], t1[:], t2[:])

            # ---------------- woven schedule ----------------
            for tci in range(NTC):
                # batch-0 attention only needs chunks <= tci-1: queue it
                # ahead of this chunk's matmuls so it fills the PSUM-
                # turnover bubble at the chunk boundary
                if 1 <= tci <= 4:
                    attn_unit(0, tci - 1, "00" if tci <= 2 else "01",
                              0 if tci % 2 else QCH)
                    if tci == 2:
                        fire_ag("00")
                    elif tci == 4:
                        fire_ag("01")
                chunk(tci)
                if tci == 3:
                    # out-proj weights, needed from the tci==5 weave onwards
                    for k4 in range(KC // 4):
                        dst = wo_sb[:, k4 * 4 * DQ:(k4 + 1) * 4 * DQ]
                        dst = dst.rearrange("p (k c) -> p k c", k=4)
                        src = woT[k4 * 4 * P:(k4 + 1) * 4 * P, :]
                        src = src.rearrange("(k p) c -> p k c", p=P)
                        nc.sync.dma_start(out=dst, in_=src)
                if tci == 4:
                    attn_unit(1, 0, "10", 0)
                elif tci == 5:
                    attn_unit(1, 1, "10", QCH)
                    fire_ag("10")
                    outproj_unit("00", 0, 0, QCH, "a")
                elif tci == 6:
                    attn_unit(1, 2, "12", 0)
                    fire_ag("12")
                    outproj_unit("00", QCH, QCH, QCH, "b")
                elif tci == 7:
                    pts13 = attn_unit(1, 3, "3a", 0, jj_range=(0, 2),
                                      ret_pts=True)
                    fire_ag("3a")
                    attn_unit(1, 3, "3b", 0, jj_range=(2, 4), pts=pts13)
                    fire_ag("3b")
                    outproj_unit("01", 2 * QCH, 0, QCH, "c")
            # tail: pure out-projection, all gathers already in flight
            outproj_unit("01", 3 * QCH, QCH, QCH, "d")
            outproj_unit("10", 4 * QCH, 0, QCH, "e")
            outproj_unit("10", 5 * QCH, QCH, QCH, "f")
            outproj_unit("12", 6 * QCH, 0, QCH, "g")
            outproj_unit("3a", 7 * QCH, 0, QCH // 2, "h", split_q=True)
            outproj_unit("3b", 7 * QCH + QCH // 2, 0, QCH // 2, "i",
                         split_q=True)

            free_sro(); free_cro(); free_wv(); free_wk(); free_wq()

    nc.compile()
    return nc


def _host_prep(inputs):
    x = np.ascontiguousarray(np.asarray(inputs["x"], np.float32).reshape(NT, D))
    wq = np.asarray(inputs["wq"], np.float32)
    wk = np.asarray(inputs["wk"], np.float32)
    wv = np.asarray(inputs["wv"], np.float32)
    wo = np.asarray(inputs["wo"], np.float32)
    cos = np.asarray(inputs["freqs_cos"], np.float32)
    sin = np.asarray(inputs["freqs_sin"], np.float32)
    mask = np.asarray(inputs["mask"], np.float32).reshape(S, S)

    tril = np.tril(np.ones((S, S), bool))
    if not mask.any():
        mode = "zeros"
    elif (mask[tril] == 0).all() and (mask[~tril] <= -1e8).all():
        mode = "causal"
    else:
        mode = "general"

    xT = np.ascontiguousarray(x.T.astype(BF))
    C = np.empty((P, S), np.float32)
    Sn = np.empty((P, S), np.float32)
    C[0::2] = cos.T
    C[1::2] = cos.T
    Sn[0::2] = -sin.T
    Sn[1::2] = sin.T
    cro = np.ascontiguousarray(C.astype(BF))
    sro = np.ascontiguousarray(Sn.astype(BF))

    cst = np.zeros((P, 3 * P), np.float32)
    cst[:, 0:P] = np.eye(P)
    pr = np.zeros((P, P), np.float32)
    idx = np.arange(0, P, 2)
    pr[idx, idx + 1] = 1.0
    pr[idx + 1, idx] = 1.0
    cst[:, P:2 * P] = pr
    cst[:, 2 * P:3 * P] = np.triu(np.ones((P, P), np.float32))
    cst = np.ascontiguousarray(cst.astype(BF))

    in_maps = []
    for c in range(N_CORES):
        r = slice(c * DQ, (c + 1) * DQ)
        m = {
            "xT": xT,
            "wqT": np.ascontiguousarray(wq[r, :].T.astype(BF)),
            "wkT": np.ascontiguousarray(wk[r, :].T.astype(BF)),
            "wvT": np.ascontiguousarray(wv[r, :].T.astype(BF)),
            "woT": np.ascontiguousarray(wo[r, :].T.astype(BF)),
            "cro": cro,
            "sro": sro,
            "cst": cst,
        }
        if mode == "general":
            m["mskT"] = np.ascontiguousarray(
                (mask.T * math.sqrt(HD)).astype(BF))
        in_maps.append(m)
    return mode, in_maps


LAST_RESULT = None


def kernel(**inputs):
    global LAST_RESULT
    from concourse.bass_utils import run_bass_kernel_spmd

    mode, in_maps = _host_prep(inputs)
    if mode not in _cache:
        _cache[mode] = _build(mode)
    nc = _cache[mode]

    res = run_bass_kernel_spmd(nc, in_maps, list(range(N_CORES)))
    LAST_RESULT = res

    out_full = np.empty((NT, D), np.float32)
    for c in range(N_CORES):
        out_full[:, c * DQ:(c + 1) * DQ] = res.results[c]["out"].T
    return out_full.reshape(B, S, D)
